# revision 1
# baseline (speedup 1.0000x reference)
"""Trainium2 Bass kernel: top-2 MoE routing (E=16, D=H=2048), 8 NeuronCores.

Strategy (memory-regime optimal: only the 2 selected experts' weights are
ever read from HBM):
  * Every core redundantly computes the gating on-device: logits = Wg@x+bg,
    top-2 indices + normalized softmax gates.
  * Weights are sharded across cores *within* each expert: core c owns rows
    [c*256, (c+1)*256) of every expert's W1 (pre-transposed to [E, D, 256])
    and the matching contraction slice of W2 (pre-transposed to
    [E, 256, H]).  After gating, each core pulls ONLY the two selected
    experts' slices (2x2MB + 2x2MB) via dynamic-offset DMAs whose expert
    index comes from a register.
  * Because the host pre-transposes the slices, the contraction index (d
    for layer 1, i for layer 2) lies on SBUF partitions, so the tensor
    engine does every matvec as accumulating [K=128, M=128, N=1] matmuls:
    h = tanh(W1[e]@x + b1[e]) lands as [128, 1] PSUM columns, which after
    tanh are directly the moving operand for layer 2.
  * Each core's gate-weighted partial output (+ tkg_k * b2[e_k]/8) is
    AllReduced across the 8 cores, yielding the exact full output.
"""

import numpy as np

try:  # make concourse importable in bare environments
    import concourse.bacc  # noqa: F401
except ImportError:  # pragma: no cover
    import sys

    sys.path.insert(0, "/opt/trn_rl_repo")

E, D, H = 16, 2048, 2048
NCORES = 8
P = 128
RS = H // NCORES  # 256 rows of each expert held per core
NCH = RS // P  # 2 partition-chunks per 256 rows
DC = D // P  # 16 contraction chunks for layer 1
OC = H // P  # 16 output chunks for layer 2

_BUILT = None


def _build(stage=2):
    """Build + compile the Bass program once. Returns (nc, input_names).

    stage: 0=gating, 1=+layer1, 2=+layer2, 3=AllReduce, 4=ReduceScatter.
    """
    global _BUILT
    if _BUILT is not None and _BUILT[2] == stage:
        return _BUILT[:2]

    import concourse.bacc as bacc
    import concourse.bass as bass
    import concourse.tile as tile
    from concourse import mybir

    f32 = mybir.dt.float32
    i32 = mybir.dt.int32
    AX = mybir.AxisListType.X
    OP = mybir.AluOpType

    nc = bacc.Bacc(
        "TRN2", target_bir_lowering=False, debug=False, num_devices=NCORES
    )

    # ----- I/O ------------------------------------------------------------
    x_d = nc.dram_tensor("x", [1, D], f32, kind="ExternalInput")
    wgt_d = nc.dram_tensor("wgt", [D, E], f32, kind="ExternalInput")  # Wg.T
    bg_d = nc.dram_tensor("bg", [1, E], f32, kind="ExternalInput")
    iota_d = nc.dram_tensor("iota16", [1, E], f32, kind="ExternalInput")
    w1t_d = b1c_d = w2t_d = b2d_d = None
    if stage >= 1:
        # W1 slice pre-transposed on host: [E, D, RS] (rows d, cols r)
        w1t_d = nc.dram_tensor("w1t", [E, D, RS], f32, kind="ExternalInput")
        b1c_d = nc.dram_tensor("b1c", [E, RS], f32, kind="ExternalInput")
    if stage >= 2:
        # W2 slice pre-transposed on host: [E, RS, H] (rows i, cols o)
        w2t_d = nc.dram_tensor("w2t", [E, RS, H], f32, kind="ExternalInput")
        b2d_d = nc.dram_tensor("b2d", [E, H], f32, kind="ExternalInput")
    out_d = nc.dram_tensor("out", [1, H], f32, kind="ExternalOutput")
    dbg_d = nc.dram_tensor("dbg", [1, 64], f32, kind="ExternalOutput")

    in_names = ["x", "wgt", "bg", "iota16"]
    if stage >= 1:
        in_names += ["w1t", "b1c"]
    if stage >= 2:
        in_names += ["w2t", "b2d"]

    with tile.TileContext(nc) as tc:
        with (
            tc.tile_pool(name="sb", bufs=1) as sb,
            tc.tile_pool(name="scr", bufs=2) as scr,
            tc.tile_pool(name="ps", bufs=1, space="PSUM") as ps,
            tc.tile_pool(name="dr", bufs=1, space="DRAM") as dr,
        ):
            # ----- static loads, spread across the three DMA rings ---------
            # x on partitions by contraction chunk: x_pd[p, dc] = x[dc*128+p]
            x_pd = sb.tile([P, DC], f32, tag="x_pd")
            nc.scalar.dma_start(
                x_pd[:], x_d.ap().rearrange("o (dc p) -> p (o dc)", p=P)
            )
            # Wg.T with contraction d on partitions: [128, dc, e]
            wgt_sb = sb.tile([P, DC * E], f32, tag="wgt")
            nc.sync.dma_start(
                wgt_sb[:].rearrange("p (dc e) -> p dc e", dc=DC),
                wgt_d.ap().rearrange("(dc p) e -> p dc e", p=P),
            )
            bg_sb = sb.tile([1, E], f32, tag="bg")
            nc.scalar.dma_start(bg_sb[:], bg_d.ap())
            iota_sb = sb.tile([1, E], f32, tag="iota")
            nc.scalar.dma_start(iota_sb[:], iota_d.ap())
            one_sb = sb.tile([1, 1], f32, tag="one")
            nc.gpsimd.memset(one_sb[:], 1.0)

            # ----- gating on PE: logits = Wg @ x + bg ----------------------
            # x chunk stationary, Wg.T chunk moving -> logits land [1, E]
            # directly in free-dim layout; bg folds in as a K=1 matmul
            lg_ps = ps.tile([1, E], f32, tag="lg_ps")
            for dc in range(DC):
                nc.tensor.matmul(
                    out=lg_ps[:],
                    lhsT=x_pd[:, dc : dc + 1],
                    rhs=wgt_sb[:, dc * E : (dc + 1) * E],
                    start=(dc == 0),
                    stop=False,
                )
            nc.tensor.matmul(
                out=lg_ps[:], lhsT=one_sb[:], rhs=bg_sb[:], start=False,
                stop=True,
            )
            logits = lg_ps

            # ----- top-1 index as fast as possible (gates the W1 DMA) ------
            m1 = sb.tile([1, 1], f32, tag="m1")
            nc.vector.tensor_reduce(m1[:], logits[:], axis=AX, op=OP.max)
            mask1 = sb.tile([1, E], f32, tag="mask1")
            nc.vector.tensor_scalar(mask1[:], logits[:], m1[:], None, OP.is_ge)
            mjunk = sb.tile([1, E], f32, tag="mjunk")
            idx1f = sb.tile([1, 1], f32, tag="idx1f")
            nc.vector.tensor_mul(mjunk[:], mask1[:], iota_sb[:])
            nc.vector.tensor_reduce(idx1f[:], mjunk[:], axis=AX, op=OP.add)
            idx_i = [
                sb.tile([1, 1], i32, tag=f"idxi{k}", name=f"idxi{k}")
                for k in range(2)
            ]
            # int index copies issue immediately after each reduce: DVE runs
            # in order, so these must NOT queue behind the tkg/debug ops --
            # they gate the register loads that start the weight DMAs
            nc.vector.tensor_copy(idx_i[0][:], idx1f[:])

            # top-2: mask out the max with a large subtraction (logits can
            # be negative, so multiplying by (1-mask) would be wrong)
            pen = sb.tile([1, E], f32, tag="pen")
            nc.vector.tensor_scalar_mul(pen[:], mask1[:], 1e30)
            l2v = sb.tile([1, E], f32, tag="l2v")
            nc.vector.tensor_tensor(
                out=l2v[:], in0=logits[:], in1=pen[:], op=OP.subtract
            )
            m2 = sb.tile([1, 1], f32, tag="m2")
            nc.vector.tensor_reduce(m2[:], l2v[:], axis=AX, op=OP.max)
            mask2 = sb.tile([1, E], f32, tag="mask2")
            nc.vector.tensor_scalar(mask2[:], l2v[:], m2[:], None, OP.is_ge)
            mjunk2 = sb.tile([1, E], f32, tag="mjunk2")
            idx2f = sb.tile([1, 1], f32, tag="idx2f")
            nc.vector.tensor_mul(mjunk2[:], mask2[:], iota_sb[:])
            nc.vector.tensor_reduce(idx2f[:], mjunk2[:], axis=AX, op=OP.add)
            nc.vector.tensor_copy(idx_i[1][:], idx2f[:])

            # normalized top-2 gates: with e_k = exp(l_k - m1):
            #   e_top1 = 1;  e_top2 = exp(m2 - m1);  S = sum(exp(logits-m1))
            #   tkg_0 = 1/(1 + e_top2 + 1e-6*S);  tkg_1 = e_top2 * tkg_0
            negm1 = sb.tile([1, 1], f32, tag="negm1")
            nc.vector.tensor_scalar_mul(negm1[:], m1[:], -1.0)
            esb = sb.tile([1, E], f32, tag="esb")
            nc.scalar.activation(
                esb[:], logits[:], mybir.ActivationFunctionType.Exp,
                bias=negm1[:],
            )
            ssum = sb.tile([1, 1], f32, tag="ssum")
            nc.vector.tensor_reduce(ssum[:], esb[:], axis=AX, op=OP.add)
            e2x = sb.tile([1, 1], f32, tag="e2x")
            nc.scalar.activation(
                e2x[:], m2[:], mybir.ActivationFunctionType.Exp, bias=negm1[:]
            )
            den = sb.tile([1, 1], f32, tag="den")
            nc.vector.tensor_scalar(den[:], ssum[:], 1e-6, 1.0, OP.mult, OP.add)
            nc.vector.tensor_add(den[:], den[:], e2x[:])
            tkg = [
                sb.tile([1, 1], f32, tag=f"tkg{k}", name=f"tkg{k}")
                for k in range(2)
            ]
            nc.vector.reciprocal(tkg[0][:], den[:])
            nc.vector.tensor_mul(tkg[1][:], e2x[:], tkg[0][:])
            tkg_rep = [
                sb.tile([P, 1], f32, tag=f"tkgr{k}", name=f"tkgr{k}")
                for k in range(2)
            ]
            for k in range(2):
                nc.gpsimd.partition_broadcast(tkg_rep[k][:], tkg[k][:])

            # debug output: logits, e, tkg, idx
            dbg = sb.tile([1, 64], f32, tag="dbg")
            nc.gpsimd.memset(dbg[:], 0.0)
            nc.vector.tensor_copy(dbg[:, 0:E], logits[:])
            nc.vector.tensor_copy(dbg[:, E : 2 * E], esb[:])
            nc.vector.tensor_copy(dbg[:, 32:33], tkg[0][:])
            nc.vector.tensor_copy(dbg[:, 33:34], tkg[1][:])
            nc.vector.tensor_copy(dbg[:, 34:35], idx1f[:])
            nc.vector.tensor_copy(dbg[:, 35:36], idx2f[:])
            nc.scalar.dma_start(dbg_d.ap(), dbg[:])

            if stage == 0:
                nc.scalar.dma_start(out_d.ap()[:, 0:DC], x_pd[0:1, :])
                nc.scalar.dma_start(
                    out_d.ap()[:, DC:], wgt_sb[0:1, 0 : D - DC]
                )

            if stage >= 1:
                # expert indices -> registers on SP (weights) + Pool (biases)
                sv = []  # SP-register index per expert slot
                pv = []  # Pool-register index per expert slot
                b1t = [
                    sb.tile([P, NCH], f32, tag=f"b1t{k}", name=f"b1t{k}")
                    for k in range(2)
                ]
                w1t = [
                    sb.tile([P, DC * RS], f32, tag=f"w1t{k}", name=f"w1t{k}")
                    for k in range(2)
                ]
                for k in range(2):
                    # expert-k index registers, then its W1 DMA immediately so
                    # the weight stream starts before idx of slot k+1 resolves
                    reg = nc.sync.alloc_register(f"idx_sp{k}")
                    nc.sync.reg_load(reg, idx_i[k][:])
                    sv.append(nc.snap(reg, donate=True, min_val=0, max_val=E - 1))
                    preg = nc.gpsimd.alloc_register(f"idx_pool{k}")
                    nc.gpsimd.reg_load(preg, idx_i[k][:])
                    pv.append(nc.snap(preg, donate=True, min_val=0, max_val=E - 1))
                    nc.sync.dma_start(
                        w1t[k][:].rearrange("p (dc r) -> p dc r", dc=DC),
                        w1t_d.ap()[bass.ds(sv[k], 1), :, :].rearrange(
                            "o (dc p) r -> p (o dc) r", p=P
                        ),
                    )
                    nc.gpsimd.dma_start(
                        b1t[k][:],
                        b1c_d.ap()[bass.ds(pv[k], 1), :].rearrange(
                            "o (c p) -> p (o c)", p=P
                        ),
                    )

                # layer 1 on PE: h_ps[:, rc] += w1t[e][:, dc, rc*128:...].T @ x
                h_ps = [
                    ps.tile([P, NCH], f32, tag=f"hps{k}", name=f"hps{k}")
                    for k in range(2)
                ]
                hs = [
                    sb.tile([P, NCH], f32, tag=f"hs{k}", name=f"hs{k}")
                    for k in range(2)
                ]
                for k in range(2):
                    for rc in range(NCH):
                        for dc in range(DC):
                            nc.tensor.matmul(
                                out=h_ps[k][:, rc : rc + 1],
                                lhsT=w1t[k][
                                    :, dc * RS + rc * P : dc * RS + (rc + 1) * P
                                ],
                                rhs=x_pd[:, dc : dc + 1],
                                start=(dc == 0),
                                stop=(dc == DC - 1),
                            )
                        nc.scalar.activation(
                            hs[k][:, rc : rc + 1],
                            h_ps[k][:, rc : rc + 1],
                            mybir.ActivationFunctionType.Tanh,
                            bias=b1t[k][:, rc : rc + 1],
                        )

            if stage == 1:
                nc.scalar.dma_start(out_d.ap()[:, 0:P], hs[0][:, 0:1])
                nc.scalar.dma_start(out_d.ap()[:, P : 2 * P], hs[0][:, 1:2])
                nc.scalar.dma_start(out_d.ap()[:, 2 * P : 3 * P], hs[1][:, 0:1])
                nc.scalar.dma_start(out_d.ap()[:, 3 * P : 4 * P], hs[1][:, 1:2])
                nc.scalar.dma_start(
                    out_d.ap()[:, 4 * P :], wgt_sb[0:1, 0 : D - 4 * P]
                )

            if stage >= 2:
                b2t = [
                    sb.tile([P, OC], f32, tag=f"b2t{k}", name=f"b2t{k}")
                    for k in range(2)
                ]
                for k in range(2):
                    nc.gpsimd.dma_start(
                        b2t[k][:],
                        b2d_d.ap()[bass.ds(pv[k], 1), :].rearrange(
                            "o (c p) -> p (o c)", p=P
                        ),
                    )
                # W2 slice (transposed): one 1MB DMA per contraction half,
                # so layer-2 matmuls on half 0 start while half 1 streams
                w2t = [
                    sb.tile([P, NCH * H], f32, tag=f"w2t{k}", name=f"w2t{k}")
                    for k in range(2)
                ]
                for k in range(2):
                    w2view = w2t_d.ap()[bass.ds(sv[k], 1), :, :].rearrange(
                        "a (ic p) o -> p (a ic) o", p=P
                    )
                    for ic in range(NCH):
                        nc.sync.dma_start(
                            w2t[k][:, ic * H : (ic + 1) * H],
                            w2view[:, ic : ic + 1, :],
                        )

                # layer 2 on PE: eo_ps[:, oc] += w2t[e][ic][:, oc*128:...].T @ h
                eo_ps = [
                    ps.tile([P, OC], f32, tag=f"eops{k}", name=f"eops{k}")
                    for k in range(2)
                ]
                eo = [
                    sb.tile([P, OC], f32, tag=f"eo{k}", name=f"eo{k}")
                    for k in range(2)
                ]
                for k in range(2):
                    for oc in range(OC):
                        for ic in range(NCH):
                            nc.tensor.matmul(
                                out=eo_ps[k][:, oc : oc + 1],
                                lhsT=w2t[k][
                                    :, ic * H + oc * P : ic * H + (oc + 1) * P
                                ],
                                rhs=hs[k][:, ic : ic + 1],
                                start=(ic == 0),
                                stop=(ic == NCH - 1),
                            )
                    nc.vector.tensor_copy(eo[k][:], eo_ps[k][:])

                # combine: res = sum_k tkg_k * (eo_k + b2[e_k]/NCORES)
                res = sb.tile([P, OC], f32, tag="res")
                vk = [
                    sb.tile([P, OC], f32, tag=f"vk{k}", name=f"vk{k}")
                    for k in range(2)
                ]
                for k in range(2):
                    nc.vector.tensor_add(vk[k][:], eo[k][:], b2t[k][:])
                    nc.vector.tensor_scalar(
                        vk[k][:], vk[k][:], tkg_rep[k][:], None, OP.mult
                    )
                nc.vector.tensor_add(res[:], vk[0][:], vk[1][:])

            if stage == 2:
                nc.sync.dma_start(
                    out_d.ap().rearrange("o (oc p) -> (o p) oc", p=P), res[:]
                )

            if stage == 3:
                cc_in = dr.tile([1, H], f32, tag="cc_in")
                cc_out = dr.tile([1, H], f32, tag="cc_out")
                nc.sync.dma_start(
                    cc_in[:].rearrange("o (oc p) -> (o p) oc", p=P), res[:]
                )
                nc.gpsimd.collective_compute(
                    "AllReduce",
                    mybir.AluOpType.add,
                    replica_groups=[list(range(NCORES))],
                    ins=[cc_in[:]],
                    outs=[cc_out[:]],
                )
                nc.sync.dma_start(out_d.ap(), cc_out[:])

            if stage >= 4:
                # ReduceScatter: core c receives the summed elements
                # [c*RS, (c+1)*RS); the host concatenates the 8 shards.
                cc_in = dr.tile([1, H], f32, tag="cc_in")
                cc_sh = dr.tile([1, RS], f32, tag="cc_sh")
                nc.sync.dma_start(
                    cc_in[:].rearrange("o (oc p) -> (o p) oc", p=P), res[:]
                )
                nc.gpsimd.collective_compute(
                    "ReduceScatter",
                    mybir.AluOpType.add,
                    replica_groups=[list(range(NCORES))],
                    ins=[cc_in[:]],
                    outs=[cc_sh[:]],
                )
                nc.sync.dma_start(out_d.ap()[:, 0:RS], cc_sh[:])

    nc.compile()
    _BUILT = (nc, in_names, stage)
    return _BUILT[:2]


def make_in_maps(x, Wg, bg, W1, b1, W2, b2):
    """Host-side sharding: per-core input dicts."""
    x = np.ascontiguousarray(np.asarray(x, np.float32).reshape(1, D))
    Wg = np.asarray(Wg, np.float32)
    bg = np.ascontiguousarray(np.asarray(bg, np.float32).reshape(1, E))
    W1 = np.asarray(W1, np.float32)
    b1 = np.asarray(b1, np.float32)
    W2 = np.asarray(W2, np.float32)
    b2 = np.asarray(b2, np.float32)

    wgt = np.ascontiguousarray(Wg.T)
    b2d = np.ascontiguousarray(b2 / NCORES)
    iota16 = np.arange(E, dtype=np.float32).reshape(1, E)

    in_maps = []
    for c in range(NCORES):
        rs = slice(c * RS, (c + 1) * RS)
        in_maps.append(
            {
                "x": x,
                "wgt": wgt,
                "bg": bg,
                "w1t": np.ascontiguousarray(W1[:, rs, :].transpose(0, 2, 1)),
                "b1c": np.ascontiguousarray(b1[:, rs]),
                "w2t": np.ascontiguousarray(W2[:, :, rs].transpose(0, 2, 1)),
                "b2d": b2d,
                "iota16": iota16,
            }
        )
    return in_maps


def kernel(x, Wg, bg, W1, b1, W2, b2, train=0, **_unused):
    import os

    from concourse import bass_utils

    stage = int(os.environ.get("MOE_STAGE", "2"))
    nc, _ = _build(stage=stage)
    in_maps = make_in_maps(x, Wg, bg, W1, b1, W2, b2)
    res = bass_utils.run_bass_kernel_spmd(
        nc, in_maps, core_ids=list(range(NCORES))
    )
    outs = [
        np.asarray(res.results[c]["out"], np.float32).reshape(H)
        for c in range(NCORES)
    ]
    if stage == 2:
        # each core holds the gate-weighted partial sum over its contraction
        # shard; unshard by summing the partials
        return np.sum(outs, axis=0, dtype=np.float32)
    if stage == 3:  # AllReduce: every core has the full output
        return outs[0]
    # stage 4, ReduceScatter: core c holds rows [c*RS, (c+1)*RS)
    return np.concatenate([o[0:RS] for o in outs])



# revision 3
# speedup vs baseline: 2.4075x; 2.4075x over previous
"""Trainium2 Bass kernel: top-2 MoE routing (E=16, D=H=2048), 8 NeuronCores.

v2 strategy (memory regime -- minimize dynamic HBM bytes + latency):
  * Host pre-quantizes weights: W1 slices to fp8-e3m4 (4-bit mantissa,
    scale s1 folded into x), W2 slices to e3m4 (scale s2 folded into the
    normalized gates) or bf16 -- cutting the per-core dynamic weight stream
    from 8MB fp32 to ~1-1.5MB.
  * Gating runs twice on PE: a bf16 fast path that feeds DVE max/max_index
    (top-2 indices in 2 ops) to start the weight DMAs ASAP, and an fp32
    path (off the critical path) for exact softmax gate values.
  * Expert weights are fetched with register-indexed DMAs (idx -> SP regs).
    b1 rides inside the W1 block as an extra K=1 matmul chunk; b2 is
    gathered by Pool SWDGE DMAs and folded in via a precomputed bcomb tile.
  * Layer 1: h = tanh(W1[e] @ x + b1[e]) as accumulating [128,128]x[128,1]
    matmuls; h is scaled by tkg_e/s2 so layer 2 accumulates gate-weighted
    outputs directly. Tail is one DVE add (eo0+bcomb, early) + one DVE add
    (+eo1) + a 64B-row output DMA.
  * Each core owns rows [c*256,(c+1)*256) of every expert's W1 and the
    matching contraction slice of W2; host sums the 8 partial outputs.
"""

import numpy as np

try:  # make concourse importable in bare environments
    import concourse.bacc  # noqa: F401
except ImportError:  # pragma: no cover
    import sys

    sys.path.insert(0, "/opt/trn_rl_repo")

import ml_dtypes

E, D, H = 16, 2048, 2048
NCORES = 8
P = 128
RS = H // NCORES  # 256 rows of each expert held per core
NCH = RS // P  # 2 partition-chunks per 256 rows
DC = D // P  # 16 contraction chunks for layer 1
OC = H // P  # 16 output chunks for layer 2

S1 = 100.0  # W1 quant scale (folded into xl1)
S2 = 100.0  # W2 quant scale (folded into tkg)
W2_BF16 = False  # True: W2 in bf16 (safer numerics, +3us stream)

F1 = DC * RS + NCH * P  # 4352 cols of w1q (incl. bias chunk)
F2 = NCH * H  # 4096 cols of w2q
F2_SPLIT = (NCH - 1) * H + (OC - 2) * P  # 3840: small tail chunk

_BUILT = None


def _build():
    """Build + compile the Bass program once. Returns (nc, input_names)."""
    global _BUILT
    if _BUILT is not None:
        return _BUILT

    import concourse.bacc as bacc
    import concourse.bass as bass
    import concourse.tile as tile
    from concourse import mybir

    f32 = mybir.dt.float32
    bf16 = mybir.dt.bfloat16
    f8 = mybir.dt.float8e3
    u32 = mybir.dt.uint32
    w2dt = bf16 if W2_BF16 else f8
    AX = mybir.AxisListType.X
    OP = mybir.AluOpType
    ACTF = mybir.ActivationFunctionType

    nc = bacc.Bacc(
        "TRN2", target_bir_lowering=False, debug=False, num_devices=NCORES
    )

    # ----- I/O --------------------------------------------------------------
    # gpbf/gpf: [x_pd(16) | WgT(256) | bg(16 on row0)] in bf16 / f32
    gpbf_d = nc.dram_tensor("gpbf", [P, 288], bf16, kind="ExternalInput")
    gpf_d = nc.dram_tensor("gpf", [P, 288], f32, kind="ExternalInput")
    # xl1: cols 0..15 = x/S1, col 16 = 1.0 (bias matmul rhs)
    xl1_d = nc.dram_tensor("xl1", [P, 17], bf16, kind="ExternalInput")
    # w1q: per expert [128p, dc*256+j] = W1[e, c*256+j, dc*128+p]*S1 (e3m4),
    #      cols 4096..4351 = b1 chunk on partition 0 only
    w1q_d = nc.dram_tensor("w1q", [E, P, F1], f8, kind="ExternalInput")
    # w2q: per expert [128p, ic*2048+o] = W2[e, o, c*256+ic*128+p]*S2
    w2q_d = nc.dram_tensor("w2q", [E, P, F2], w2dt, kind="ExternalInput")
    # b2c: [e, p, oc] = b2[e, oc*128+p] * S2 / NCORES
    b2c_d = nc.dram_tensor("b2c", [E, P, OC], f32, kind="ExternalInput")
    out_d = nc.dram_tensor("out", [P, OC], f32, kind="ExternalOutput")

    in_names = ["gpbf", "gpf", "xl1", "w1q", "w2q", "b2c"]

    with tile.TileContext(nc) as tc:
        with (
            tc.tile_pool(name="sb", bufs=1) as sb,
            tc.tile_pool(name="ps", bufs=1, space="PSUM") as ps,
        ):
            # ----- static loads --------------------------------------------
            gp_bf = sb.tile([P, 288], bf16, tag="gp_bf")
            nc.sync.dma_start(gp_bf[:], gpbf_d.ap())  # SP: critical path
            gp_f = sb.tile([P, 288], f32, tag="gp_f")
            nc.scalar.dma_start(gp_f[:], gpf_d.ap())
            xl1 = sb.tile([P, 17], bf16, tag="xl1")
            nc.scalar.dma_start(xl1[:], xl1_d.ap())

            one_bf = sb.tile([1, 1], bf16, tag="one_bf")
            nc.gpsimd.memset(one_bf[:], 1.0)
            one_f = sb.tile([1, 1], f32, tag="one_f")
            nc.gpsimd.memset(one_f[:], 1.0)
            ones_col = sb.tile([1, P], f32, tag="ones_col")
            nc.gpsimd.memset(ones_col[:], 1.0)

            # ----- gating fast path (bf16) on PE ---------------------------
            lg_bf = ps.tile([1, E], f32, tag="lg_bf")
            for dc in range(DC):
                nc.tensor.matmul(
                    out=lg_bf[:],
                    lhsT=gp_bf[:, dc : dc + 1],
                    rhs=gp_bf[:, 16 + dc * E : 16 + (dc + 1) * E],
                    start=(dc == 0),
                    stop=False,
                )
            nc.tensor.matmul(
                out=lg_bf[:], lhsT=one_bf[:], rhs=gp_bf[0:1, 272:288],
                start=False, stop=True,
            )

            # top-2 indices in two DVE ops
            vmax_bf = sb.tile([1, 8], f32, tag="vmax_bf")
            nc.vector.max(vmax_bf[:], lg_bf[:])
            vidx = sb.tile([1, 8], u32, tag="vidx")
            nc.vector.max_index(vidx[:], vmax_bf[:], lg_bf[:])

            # ----- idx -> SP registers -> dynamic weight DMAs ---------------
            sv = []
            for k in range(2):
                reg = nc.sync.alloc_register(f"idx_sp{k}")
                nc.sync.reg_load(reg, vidx[0:1, k : k + 1])
                sv.append(nc.snap(reg, donate=True, min_val=0, max_val=E - 1))

            w1t = [
                sb.tile([P, F1], f8, tag=f"w1t{k}", name=f"w1t{k}")
                for k in range(2)
            ]
            w2t = [
                sb.tile([P, F2], w2dt, tag=f"w2t{k}", name=f"w2t{k}")
                for k in range(2)
            ]
            for k in range(2):
                nc.sync.dma_start(
                    w1t[k][:],
                    w1q_d.ap()[bass.ds(sv[k], 1), :, :].rearrange(
                        "o p f -> p (o f)"
                    ),
                )
            nc.sync.dma_start(
                w2t[0][:],
                w2q_d.ap()[bass.ds(sv[0], 1), :, :].rearrange(
                    "o p f -> p (o f)"
                ),
            )
            # e1's W2 split so the last DMA chunk (oc 14..15 of ic 1) is small
            nc.sync.dma_start(
                w2t[1][:, 0:F2_SPLIT],
                w2q_d.ap()[bass.ds(sv[1], 1), :, 0:F2_SPLIT].rearrange(
                    "o p f -> p (o f)"
                ),
            )
            nc.sync.dma_start(
                w2t[1][:, F2_SPLIT:F2],
                w2q_d.ap()[bass.ds(sv[1], 1), :, F2_SPLIT:F2].rearrange(
                    "o p f -> p (o f)"
                ),
            )

            # ----- b2 gather on Pool (SWDGE; off critical path) -------------
            pv = []
            for k in range(2):
                preg = nc.gpsimd.alloc_register(f"idx_pool{k}")
                nc.gpsimd.reg_load(preg, vidx[0:1, k : k + 1])
                pv.append(nc.snap(preg, donate=True, min_val=0, max_val=E - 1))
            b2t = [
                sb.tile([P, OC], f32, tag=f"b2t{k}", name=f"b2t{k}")
                for k in range(2)
            ]
            for k in range(2):
                nc.gpsimd.dma_start(
                    b2t[k][:],
                    b2c_d.ap()[bass.ds(pv[k], 1), :, :].rearrange(
                        "o p f -> p (o f)"
                    ),
                )

            # ----- gating slow path (fp32) for exact gate values ------------
            lg_f = ps.tile([1, E], f32, tag="lg_f")
            for dc in range(DC):
                nc.tensor.matmul(
                    out=lg_f[:],
                    lhsT=gp_f[:, dc : dc + 1],
                    rhs=gp_f[:, 16 + dc * E : 16 + (dc + 1) * E],
                    start=(dc == 0),
                    stop=False,
                )
            nc.tensor.matmul(
                out=lg_f[:], lhsT=one_f[:], rhs=gp_f[0:1, 272:288],
                start=False, stop=True,
            )

            # tkg'_k = tkg_k / S2, with tkg = top2(softmax)/(sum+1e-6)
            vmax_f = sb.tile([1, 8], f32, tag="vmax_f")
            nc.vector.max(vmax_f[:], lg_f[:])
            negm1 = sb.tile([1, 1], f32, tag="negm1")
            nc.vector.tensor_scalar_mul(negm1[:], vmax_f[0:1, 0:1], -1.0)
            esb = sb.tile([1, E], f32, tag="esb")
            nc.scalar.activation(esb[:], lg_f[:], ACTF.Exp, bias=negm1[:])
            e2x = sb.tile([1, 1], f32, tag="e2x")
            nc.scalar.activation(
                e2x[:], vmax_f[0:1, 1:2], ACTF.Exp, bias=negm1[:]
            )
            ssum = sb.tile([1, 1], f32, tag="ssum")
            nc.vector.tensor_reduce(ssum[:], esb[:], axis=AX, op=OP.add)
            # den2 = S2 * (1 + e2x + 1e-6*ssum)
            den2 = sb.tile([1, 1], f32, tag="den2")
            nc.vector.tensor_scalar(
                den2[:], ssum[:], 1e-6 * S2, S2, OP.mult, OP.add
            )
            e2xs = sb.tile([1, 1], f32, tag="e2xs")
            nc.vector.tensor_scalar_mul(e2xs[:], e2x[:], S2)
            nc.vector.tensor_add(den2[:], den2[:], e2xs[:])
            tkgp = [
                sb.tile([1, 1], f32, tag=f"tkgp{k}", name=f"tkgp{k}")
                for k in range(2)
            ]
            nc.vector.reciprocal(tkgp[0][:], den2[:])
            nc.vector.tensor_mul(tkgp[1][:], e2x[:], tkgp[0][:])

            # broadcast tkg' to all partitions via K=1 matmuls
            tkgrep = ps.tile([P, 2], f32, tag="tkgrep")
            for k in range(2):
                nc.tensor.matmul(
                    out=tkgrep[:, k : k + 1],
                    lhsT=ones_col[:],
                    rhs=tkgp[k][:],
                    start=True,
                    stop=True,
                )

            # ----- layer 1: h = tanh(W1[e] @ x + b1[e]), scaled by tkg' -----
            h_ps = [
                ps.tile([P, NCH], f32, tag=f"hps{k}", name=f"hps{k}")
                for k in range(2)
            ]
            hs = [
                sb.tile([P, NCH], bf16, tag=f"hs{k}", name=f"hs{k}")
                for k in range(2)
            ]
            for k in range(2):
                for rc in range(NCH):
                    for dc in range(DC):
                        nc.tensor.matmul(
                            out=h_ps[k][:, rc : rc + 1],
                            lhsT=w1t[k][
                                :, dc * RS + rc * P : dc * RS + (rc + 1) * P
                            ],
                            rhs=xl1[:, dc : dc + 1],
                            start=(dc == 0),
                            stop=False,
                        )
                    # b1 chunk (partition 0 only) x 1.0
                    nc.tensor.matmul(
                        out=h_ps[k][:, rc : rc + 1],
                        lhsT=w1t[k][
                            :, DC * RS + rc * P : DC * RS + (rc + 1) * P
                        ],
                        rhs=xl1[:, 16:17],
                        start=False,
                        stop=True,
                    )
                for rc in range(NCH):
                    nc.scalar.activation(
                        hs[k][:, rc : rc + 1],
                        h_ps[k][:, rc : rc + 1],
                        ACTF.Tanh,
                    )
                nc.vector.tensor_scalar(
                    hs[k][:], hs[k][:], tkgrep[:, k : k + 1], None, OP.mult
                )

            # ----- layer 2: eo_k = W2[e_k] @ hs_k (gate-weighted) -----------
            eo_ps = [
                ps.tile([P, OC], f32, tag=f"eops{k}", name=f"eops{k}")
                for k in range(2)
            ]
            for k in range(2):
                for oc in range(OC):
                    for ic in range(NCH):
                        nc.tensor.matmul(
                            out=eo_ps[k][:, oc : oc + 1],
                            lhsT=w2t[k][
                                :, ic * H + oc * P : ic * H + (oc + 1) * P
                            ],
                            rhs=hs[k][:, ic : ic + 1],
                            start=(ic == 0),
                            stop=(ic == NCH - 1),
                        )

            # ----- combine: res = eo0 + eo1 + sum_k tkg'_k*b2c[e_k] ---------
            vb = [
                sb.tile([P, OC], f32, tag=f"vb{k}", name=f"vb{k}")
                for k in range(2)
            ]
            for k in range(2):
                nc.vector.tensor_scalar(
                    vb[k][:], b2t[k][:], tkgrep[:, k : k + 1], None, OP.mult
                )
            bcomb = sb.tile([P, OC], f32, tag="bcomb")
            nc.vector.tensor_add(bcomb[:], vb[0][:], vb[1][:])
            add1 = sb.tile([P, OC], f32, tag="add1")
            nc.vector.tensor_tensor(
                out=add1[:], in0=eo_ps[0][:], in1=bcomb[:], op=OP.add
            )
            res = sb.tile([P, OC], f32, tag="res")
            nc.vector.tensor_tensor(
                out=res[:], in0=eo_ps[1][:], in1=add1[:], op=OP.add
            )
            nc.sync.dma_start(out_d.ap(), res[:])

    nc.compile()
    _BUILT = (nc, in_names)
    return _BUILT


def make_in_maps(x, Wg, bg, W1, b1, W2, b2):
    """Host-side packing/quantization: per-core input dicts."""
    bf = ml_dtypes.bfloat16
    f8 = ml_dtypes.float8_e3m4
    x = np.asarray(x, np.float32).reshape(D)
    Wg = np.asarray(Wg, np.float32)
    bg = np.asarray(bg, np.float32).reshape(E)
    W1 = np.asarray(W1, np.float32)
    b1 = np.asarray(b1, np.float32)
    W2 = np.asarray(W2, np.float32)
    b2 = np.asarray(b2, np.float32)

    # gating pack [128, 288]: x_pd | WgT | bg(row 0)
    gp = np.zeros((P, 288), np.float32)
    gp[:, 0:DC] = x.reshape(DC, P).T
    gp[:, 16 : 16 + DC * E] = (
        Wg.T.reshape(DC, P, E).transpose(1, 0, 2).reshape(P, DC * E)
    )
    gp[0, 272 : 272 + E] = bg
    gpbf = gp.astype(bf)

    xl1 = np.zeros((P, 17), np.float32)
    xl1[:, 0:DC] = x.reshape(DC, P).T / S1
    xl1[:, 16] = 1.0
    xl1 = xl1.astype(bf)

    w2np = bf if W2_BF16 else f8

    in_maps = []
    for c in range(NCORES):
        rs = slice(c * RS, (c + 1) * RS)
        # W1 slice + bias chunk
        w1s = (
            (W1[:, rs, :] * S1)
            .reshape(E, RS, DC, P)
            .transpose(0, 3, 2, 1)
            .reshape(E, P, DC * RS)
        )
        w1b = np.zeros((E, P, NCH * P), np.float32)
        w1b[:, 0, :] = b1[:, rs]
        w1q = np.ascontiguousarray(
            np.concatenate([w1s, w1b], axis=2)
        ).astype(f8)
        # W2 slice
        w2q = np.ascontiguousarray(
            (W2[:, :, rs] * S2)
            .reshape(E, H, NCH, P)
            .transpose(0, 3, 2, 1)
            .reshape(E, P, NCH * H)
        ).astype(w2np)
        b2c = np.ascontiguousarray(
            b2.reshape(E, OC, P).transpose(0, 2, 1) * (S2 / NCORES)
        )
        in_maps.append(
            {
                "gpbf": gpbf,
                "gpf": gp,
                "xl1": xl1,
                "w1q": w1q,
                "w2q": w2q,
                "b2c": b2c,
            }
        )
    return in_maps


def kernel(x, Wg, bg, W1, b1, W2, b2, train=0, **_unused):
    from concourse import bass_utils

    nc, _ = _build()
    in_maps = make_in_maps(x, Wg, bg, W1, b1, W2, b2)
    res = bass_utils.run_bass_kernel_spmd(
        nc, in_maps, core_ids=list(range(NCORES))
    )
    outs = [
        np.asarray(res.results[c]["out"], np.float32).reshape(P, OC)
        for c in range(NCORES)
    ]
    # each core holds a gate-weighted partial over its contraction shard;
    # out[oc*128+p] = sum_c outs[c][p, oc]
    tot = np.sum(outs, axis=0, dtype=np.float32)
    return tot.T.reshape(H).copy()


# revision 4
# speedup vs baseline: 2.4105x; 1.0012x over previous
"""Trainium2 Bass kernel: top-2 MoE routing (E=16, D=H=2048), 8 NeuronCores.

v2 strategy (memory regime -- minimize dynamic HBM bytes + latency):
  * Host pre-quantizes weights: W1 slices to fp8-e3m4 (4-bit mantissa,
    scale s1 folded into x), W2 slices to e3m4 (scale s2 folded into the
    normalized gates) or bf16 -- cutting the per-core dynamic weight stream
    from 8MB fp32 to ~1-1.5MB.
  * Gating runs twice on PE: a bf16 fast path that feeds DVE max/max_index
    (top-2 indices in 2 ops) to start the weight DMAs ASAP, and an fp32
    path (off the critical path) for exact softmax gate values.
  * Expert weights are fetched with register-indexed DMAs (idx -> SP regs).
    b1 rides inside the W1 block as an extra K=1 matmul chunk; b2 is
    gathered by Pool SWDGE DMAs and folded in via a precomputed bcomb tile.
  * Layer 1: h = tanh(W1[e] @ x + b1[e]) as accumulating [128,128]x[128,1]
    matmuls; h is scaled by tkg_e/s2 so layer 2 accumulates gate-weighted
    outputs directly. Tail is one DVE add (eo0+bcomb, early) + one DVE add
    (+eo1) + a 64B-row output DMA.
  * Each core owns rows [c*256,(c+1)*256) of every expert's W1 and the
    matching contraction slice of W2; host sums the 8 partial outputs.
"""

import numpy as np

try:  # make concourse importable in bare environments
    import concourse.bacc  # noqa: F401
except ImportError:  # pragma: no cover
    import sys

    sys.path.insert(0, "/opt/trn_rl_repo")

import ml_dtypes

E, D, H = 16, 2048, 2048
NCORES = 8
P = 128
RS = H // NCORES  # 256 rows of each expert held per core
NCH = RS // P  # 2 partition-chunks per 256 rows
DC = D // P  # 16 contraction chunks for layer 1
OC = H // P  # 16 output chunks for layer 2

S1 = 100.0  # W1 quant scale (folded into xl1)
S2 = 100.0  # W2 quant scale (folded into tkg)
W2_BF16 = False  # True: W2 in bf16 (safer numerics, +3us stream)

F1 = DC * RS + NCH * P  # 4352 cols of w1q (incl. bias chunk)
F2 = NCH * H  # 4096 cols of w2q
F2_SPLIT = (NCH - 1) * H + (OC - 1) * P  # 3968: tiny last chunk (1 col)

_BUILT = None


def _build():
    """Build + compile the Bass program once. Returns (nc, input_names)."""
    global _BUILT
    if _BUILT is not None:
        return _BUILT

    import concourse.bacc as bacc
    import concourse.bass as bass
    import concourse.tile as tile
    from concourse import mybir

    f32 = mybir.dt.float32
    bf16 = mybir.dt.bfloat16
    f8 = mybir.dt.float8e3
    u32 = mybir.dt.uint32
    w2dt = bf16 if W2_BF16 else f8
    AX = mybir.AxisListType.X
    OP = mybir.AluOpType
    ACTF = mybir.ActivationFunctionType

    nc = bacc.Bacc(
        "TRN2", target_bir_lowering=False, debug=False, num_devices=NCORES
    )

    # ----- I/O --------------------------------------------------------------
    # gpbf/gpf: [x_pd(16) | WgT(256) | bg(16 on row0)] in bf16 / f32
    gpbf_d = nc.dram_tensor("gpbf", [P, 288], bf16, kind="ExternalInput")
    gpf_d = nc.dram_tensor("gpf", [P, 288], f32, kind="ExternalInput")
    # xl1: cols 0..15 = x/S1, col 16 = 1.0 (bias matmul rhs)
    xl1_d = nc.dram_tensor("xl1", [P, 17], bf16, kind="ExternalInput")
    # w1q: per expert [128p, dc*256+j] = W1[e, c*256+j, dc*128+p]*S1 (e3m4),
    #      cols 4096..4351 = b1 chunk on partition 0 only
    w1q_d = nc.dram_tensor("w1q", [E, P, F1], f8, kind="ExternalInput")
    # w2q: per expert [128p, ic*2048+o] = W2[e, o, c*256+ic*128+p]*S2
    w2q_d = nc.dram_tensor("w2q", [E, P, F2], w2dt, kind="ExternalInput")
    # b2c: [e, p, oc] = b2[e, oc*128+p] * S2 / NCORES
    b2c_d = nc.dram_tensor("b2c", [E, P, OC], f32, kind="ExternalInput")
    out_d = nc.dram_tensor("out", [P, OC], f32, kind="ExternalOutput")

    in_names = ["gpbf", "gpf", "xl1", "w1q", "w2q", "b2c"]

    with tile.TileContext(nc) as tc:
        with (
            tc.tile_pool(name="sb", bufs=1) as sb,
            tc.tile_pool(name="ps", bufs=1, space="PSUM") as ps,
        ):
            # ----- static loads --------------------------------------------
            gp_bf = sb.tile([P, 288], bf16, tag="gp_bf")
            nc.sync.dma_start(gp_bf[:], gpbf_d.ap())  # SP: critical path
            gp_f = sb.tile([P, 288], f32, tag="gp_f")
            nc.scalar.dma_start(gp_f[:], gpf_d.ap())
            xl1 = sb.tile([P, 17], bf16, tag="xl1")
            nc.scalar.dma_start(xl1[:], xl1_d.ap())

            one_bf = sb.tile([1, 1], bf16, tag="one_bf")
            nc.gpsimd.memset(one_bf[:], 1.0)
            one_f = sb.tile([1, 1], f32, tag="one_f")
            nc.gpsimd.memset(one_f[:], 1.0)
            ones_col = sb.tile([1, P], f32, tag="ones_col")
            nc.gpsimd.memset(ones_col[:], 1.0)

            # ----- gating fast path (bf16) on PE ---------------------------
            lg_bf = ps.tile([1, E], f32, tag="lg_bf")
            for dc in range(DC):
                nc.tensor.matmul(
                    out=lg_bf[:],
                    lhsT=gp_bf[:, dc : dc + 1],
                    rhs=gp_bf[:, 16 + dc * E : 16 + (dc + 1) * E],
                    start=(dc == 0),
                    stop=False,
                )
            nc.tensor.matmul(
                out=lg_bf[:], lhsT=one_bf[:], rhs=gp_bf[0:1, 272:288],
                start=False, stop=True,
            )

            # top-2 indices in two DVE ops
            vmax_bf = sb.tile([1, 8], f32, tag="vmax_bf")
            nc.vector.max(vmax_bf[:], lg_bf[:])
            vidx = sb.tile([1, 8], u32, tag="vidx")
            nc.vector.max_index(vidx[:], vmax_bf[:], lg_bf[:])

            # ----- idx -> SP registers -> dynamic weight DMAs ---------------
            sv = []
            for k in range(2):
                reg = nc.sync.alloc_register(f"idx_sp{k}")
                nc.sync.reg_load(reg, vidx[0:1, k : k + 1])
                sv.append(nc.snap(reg, donate=True, min_val=0, max_val=E - 1))

            w1t = [
                sb.tile([P, F1], f8, tag=f"w1t{k}", name=f"w1t{k}")
                for k in range(2)
            ]
            w2t = [
                sb.tile([P, F2], w2dt, tag=f"w2t{k}", name=f"w2t{k}")
                for k in range(2)
            ]
            for k in range(2):
                nc.sync.dma_start(
                    w1t[k][:],
                    w1q_d.ap()[bass.ds(sv[k], 1), :, :].rearrange(
                        "o p f -> p (o f)"
                    ),
                )
            nc.sync.dma_start(
                w2t[0][:],
                w2q_d.ap()[bass.ds(sv[0], 1), :, :].rearrange(
                    "o p f -> p (o f)"
                ),
            )
            # e1's W2 split so the last DMA chunk (oc 14..15 of ic 1) is small
            nc.sync.dma_start(
                w2t[1][:, 0:F2_SPLIT],
                w2q_d.ap()[bass.ds(sv[1], 1), :, 0:F2_SPLIT].rearrange(
                    "o p f -> p (o f)"
                ),
            )
            nc.sync.dma_start(
                w2t[1][:, F2_SPLIT:F2],
                w2q_d.ap()[bass.ds(sv[1], 1), :, F2_SPLIT:F2].rearrange(
                    "o p f -> p (o f)"
                ),
            )

            # ----- b2 gather on Pool (SWDGE; off critical path) -------------
            pv = []
            for k in range(2):
                preg = nc.gpsimd.alloc_register(f"idx_pool{k}")
                nc.gpsimd.reg_load(preg, vidx[0:1, k : k + 1])
                pv.append(nc.snap(preg, donate=True, min_val=0, max_val=E - 1))
            b2t = [
                sb.tile([P, OC], f32, tag=f"b2t{k}", name=f"b2t{k}")
                for k in range(2)
            ]
            for k in range(2):
                nc.gpsimd.dma_start(
                    b2t[k][:],
                    b2c_d.ap()[bass.ds(pv[k], 1), :, :].rearrange(
                        "o p f -> p (o f)"
                    ),
                )

            # ----- gating slow path (fp32) for exact gate values ------------
            lg_f = ps.tile([1, E], f32, tag="lg_f")
            for dc in range(DC):
                nc.tensor.matmul(
                    out=lg_f[:],
                    lhsT=gp_f[:, dc : dc + 1],
                    rhs=gp_f[:, 16 + dc * E : 16 + (dc + 1) * E],
                    start=(dc == 0),
                    stop=False,
                )
            nc.tensor.matmul(
                out=lg_f[:], lhsT=one_f[:], rhs=gp_f[0:1, 272:288],
                start=False, stop=True,
            )

            # tkg'_k = tkg_k / S2, with tkg = top2(softmax)/(sum+1e-6)
            vmax_f = sb.tile([1, 8], f32, tag="vmax_f")
            nc.vector.max(vmax_f[:], lg_f[:])
            negm1 = sb.tile([1, 1], f32, tag="negm1")
            nc.vector.tensor_scalar_mul(negm1[:], vmax_f[0:1, 0:1], -1.0)
            esb = sb.tile([1, E], f32, tag="esb")
            nc.scalar.activation(esb[:], lg_f[:], ACTF.Exp, bias=negm1[:])
            e2x = sb.tile([1, 1], f32, tag="e2x")
            nc.scalar.activation(
                e2x[:], vmax_f[0:1, 1:2], ACTF.Exp, bias=negm1[:]
            )
            ssum = sb.tile([1, 1], f32, tag="ssum")
            nc.vector.tensor_reduce(ssum[:], esb[:], axis=AX, op=OP.add)
            # den2 = S2 * (1 + e2x + 1e-6*ssum)
            den2 = sb.tile([1, 1], f32, tag="den2")
            nc.vector.tensor_scalar(
                den2[:], ssum[:], 1e-6 * S2, S2, OP.mult, OP.add
            )
            e2xs = sb.tile([1, 1], f32, tag="e2xs")
            nc.vector.tensor_scalar_mul(e2xs[:], e2x[:], S2)
            nc.vector.tensor_add(den2[:], den2[:], e2xs[:])
            tkgp = [
                sb.tile([1, 1], f32, tag=f"tkgp{k}", name=f"tkgp{k}")
                for k in range(2)
            ]
            nc.vector.reciprocal(tkgp[0][:], den2[:])
            nc.vector.tensor_mul(tkgp[1][:], e2x[:], tkgp[0][:])

            # broadcast tkg' to all partitions via K=1 matmuls
            tkgrep = ps.tile([P, 2], f32, tag="tkgrep")
            for k in range(2):
                nc.tensor.matmul(
                    out=tkgrep[:, k : k + 1],
                    lhsT=ones_col[:],
                    rhs=tkgp[k][:],
                    start=True,
                    stop=True,
                )

            # ----- layer 1: h = tanh(W1[e] @ x + b1[e]), scaled by tkg' -----
            h_ps = [
                ps.tile([P, NCH], f32, tag=f"hps{k}", name=f"hps{k}")
                for k in range(2)
            ]
            hs = [
                sb.tile([P, NCH], bf16, tag=f"hs{k}", name=f"hs{k}")
                for k in range(2)
            ]
            for k in range(2):
                for rc in range(NCH):
                    for dc in range(DC):
                        nc.tensor.matmul(
                            out=h_ps[k][:, rc : rc + 1],
                            lhsT=w1t[k][
                                :, dc * RS + rc * P : dc * RS + (rc + 1) * P
                            ],
                            rhs=xl1[:, dc : dc + 1],
                            start=(dc == 0),
                            stop=False,
                        )
                    # b1 chunk (partition 0 only) x 1.0
                    nc.tensor.matmul(
                        out=h_ps[k][:, rc : rc + 1],
                        lhsT=w1t[k][
                            :, DC * RS + rc * P : DC * RS + (rc + 1) * P
                        ],
                        rhs=xl1[:, 16:17],
                        start=False,
                        stop=True,
                    )
                for rc in range(NCH):
                    nc.scalar.activation(
                        hs[k][:, rc : rc + 1],
                        h_ps[k][:, rc : rc + 1],
                        ACTF.Tanh,
                    )
                nc.vector.tensor_scalar(
                    hs[k][:], hs[k][:], tkgrep[:, k : k + 1], None, OP.mult
                )

            # ----- layer 2: eo_k = W2[e_k] @ hs_k (gate-weighted) -----------
            eo_ps = [
                ps.tile([P, OC], f32, tag=f"eops{k}", name=f"eops{k}")
                for k in range(2)
            ]
            for k in range(2):
                for oc in range(OC):
                    for ic in range(NCH):
                        nc.tensor.matmul(
                            out=eo_ps[k][:, oc : oc + 1],
                            lhsT=w2t[k][
                                :, ic * H + oc * P : ic * H + (oc + 1) * P
                            ],
                            rhs=hs[k][:, ic : ic + 1],
                            start=(ic == 0),
                            stop=(ic == NCH - 1),
                        )

            # ----- combine: res = eo0 + eo1 + sum_k tkg'_k*b2c[e_k] ---------
            vb = [
                sb.tile([P, OC], f32, tag=f"vb{k}", name=f"vb{k}")
                for k in range(2)
            ]
            for k in range(2):
                nc.vector.tensor_scalar(
                    vb[k][:], b2t[k][:], tkgrep[:, k : k + 1], None, OP.mult
                )
            bcomb = sb.tile([P, OC], f32, tag="bcomb")
            nc.vector.tensor_add(bcomb[:], vb[0][:], vb[1][:])
            add1 = sb.tile([P, OC], f32, tag="add1")
            nc.vector.tensor_tensor(
                out=add1[:], in0=eo_ps[0][:], in1=bcomb[:], op=OP.add
            )
            res = sb.tile([P, OC], f32, tag="res")
            nc.vector.tensor_tensor(
                out=res[:], in0=eo_ps[1][:], in1=add1[:], op=OP.add
            )
            nc.sync.dma_start(out_d.ap(), res[:])

    nc.compile()
    _BUILT = (nc, in_names)
    return _BUILT


def make_in_maps(x, Wg, bg, W1, b1, W2, b2):
    """Host-side packing/quantization: per-core input dicts."""
    bf = ml_dtypes.bfloat16
    f8 = ml_dtypes.float8_e3m4
    x = np.asarray(x, np.float32).reshape(D)
    Wg = np.asarray(Wg, np.float32)
    bg = np.asarray(bg, np.float32).reshape(E)
    W1 = np.asarray(W1, np.float32)
    b1 = np.asarray(b1, np.float32)
    W2 = np.asarray(W2, np.float32)
    b2 = np.asarray(b2, np.float32)

    # gating pack [128, 288]: x_pd | WgT | bg(row 0)
    gp = np.zeros((P, 288), np.float32)
    gp[:, 0:DC] = x.reshape(DC, P).T
    gp[:, 16 : 16 + DC * E] = (
        Wg.T.reshape(DC, P, E).transpose(1, 0, 2).reshape(P, DC * E)
    )
    gp[0, 272 : 272 + E] = bg
    gpbf = gp.astype(bf)

    xl1 = np.zeros((P, 17), np.float32)
    xl1[:, 0:DC] = x.reshape(DC, P).T / S1
    xl1[:, 16] = 1.0
    xl1 = xl1.astype(bf)

    w2np = bf if W2_BF16 else f8

    in_maps = []
    for c in range(NCORES):
        rs = slice(c * RS, (c + 1) * RS)
        # W1 slice + bias chunk
        w1s = (
            (W1[:, rs, :] * S1)
            .reshape(E, RS, DC, P)
            .transpose(0, 3, 2, 1)
            .reshape(E, P, DC * RS)
        )
        w1b = np.zeros((E, P, NCH * P), np.float32)
        w1b[:, 0, :] = b1[:, rs]
        w1q = np.ascontiguousarray(
            np.concatenate([w1s, w1b], axis=2)
        ).astype(f8)
        # W2 slice
        w2q = np.ascontiguousarray(
            (W2[:, :, rs] * S2)
            .reshape(E, H, NCH, P)
            .transpose(0, 3, 2, 1)
            .reshape(E, P, NCH * H)
        ).astype(w2np)
        b2c = np.ascontiguousarray(
            b2.reshape(E, OC, P).transpose(0, 2, 1) * (S2 / NCORES)
        )
        in_maps.append(
            {
                "gpbf": gpbf,
                "gpf": gp,
                "xl1": xl1,
                "w1q": w1q,
                "w2q": w2q,
                "b2c": b2c,
            }
        )
    return in_maps


def kernel(x, Wg, bg, W1, b1, W2, b2, train=0, **_unused):
    from concourse import bass_utils

    nc, _ = _build()
    in_maps = make_in_maps(x, Wg, bg, W1, b1, W2, b2)
    res = bass_utils.run_bass_kernel_spmd(
        nc, in_maps, core_ids=list(range(NCORES))
    )
    outs = [
        np.asarray(res.results[c]["out"], np.float32).reshape(P, OC)
        for c in range(NCORES)
    ]
    # each core holds a gate-weighted partial over its contraction shard;
    # out[oc*128+p] = sum_c outs[c][p, oc]
    tot = np.sum(outs, axis=0, dtype=np.float32)
    return tot.T.reshape(H).copy()


# revision 13
# speedup vs baseline: 2.4536x; 1.0179x over previous
"""Trainium2 Bass kernel: top-2 MoE routing (E=16, D=H=2048), 8 NeuronCores.

v2 strategy (memory regime -- minimize dynamic HBM bytes + latency):
  * Host pre-quantizes weights: W1 slices to fp8-e3m4 (4-bit mantissa,
    scale s1 folded into x), W2 slices to e3m4 (scale s2 folded into the
    normalized gates) or bf16 -- cutting the per-core dynamic weight stream
    from 8MB fp32 to ~1-1.5MB.
  * Gating runs twice on PE: a bf16 fast path that feeds DVE max/max_index
    (top-2 indices in 2 ops) to start the weight DMAs ASAP, and an fp32
    path (off the critical path) for exact softmax gate values.
  * Expert weights are fetched with register-indexed DMAs (idx -> SP regs).
    b1 rides inside the W1 block as an extra K=1 matmul chunk; b2 is
    gathered by Pool SWDGE DMAs and folded in via a precomputed bcomb tile.
  * Layer 1: h = tanh(W1[e] @ x + b1[e]) as accumulating [128,128]x[128,1]
    matmuls; h is scaled by tkg_e/s2 so layer 2 accumulates gate-weighted
    outputs directly. Tail is one DVE add (eo0+bcomb, early) + one DVE add
    (+eo1) + a 64B-row output DMA.
  * Each core owns rows [c*256,(c+1)*256) of every expert's W1 and the
    matching contraction slice of W2; host sums the 8 partial outputs.
"""

import numpy as np

try:  # make concourse importable in bare environments
    import concourse.bacc  # noqa: F401
except ImportError:  # pragma: no cover
    import sys

    sys.path.insert(0, "/opt/trn_rl_repo")

import ml_dtypes

E, D, H = 16, 2048, 2048
NCORES = 8
P = 128
RS = H // NCORES  # 256 rows of each expert held per core
NCH = RS // P  # 2 partition-chunks per 256 rows
DC = D // P  # 16 contraction chunks for layer 1
OC = H // P  # 16 output chunks for layer 2

S1 = 100.0  # W1 quant scale (folded into xl1)
S2 = 100.0  # W2 quant scale (folded into tkg)
W2_BF16 = False  # True: W2 in bf16 (safer numerics, +3us stream)

F1 = DC * RS + NCH * P  # 4352 cols of w1q (incl. bias chunk)
F2 = NCH * H  # 4096 cols of w2q
F2_SPLIT = (NCH - 1) * H + (OC - 1) * P  # 3968: tiny last chunk (1 col)

_BUILT = None


def _build():
    """Build + compile the Bass program once. Returns (nc, input_names)."""
    global _BUILT
    if _BUILT is not None:
        return _BUILT

    import concourse.bacc as bacc
    import concourse.bass as bass
    import concourse.tile as tile
    from concourse import mybir

    f32 = mybir.dt.float32
    bf16 = mybir.dt.bfloat16
    f8 = mybir.dt.float8e3
    u32 = mybir.dt.uint32
    w2dt = bf16 if W2_BF16 else f8
    AX = mybir.AxisListType.X
    OP = mybir.AluOpType
    ACTF = mybir.ActivationFunctionType

    nc = bacc.Bacc(
        "TRN2", target_bir_lowering=False, debug=False, num_devices=NCORES
    )

    # ----- I/O --------------------------------------------------------------
    # gpbf: [x_pd(16) | WgT(256) | bg(16 on row0)] in bf16
    gpbf_d = nc.dram_tensor("gpbf", [P, 288], bf16, kind="ExternalInput")
    # gpf: same in f32, plus col 288 = partition iota (rows 0..15)
    gpf_d = nc.dram_tensor("gpf", [P, 289], f32, kind="ExternalInput")
    # xl1: cols 0..15 = x/S1, col 16 = 1.0 (bias matmul rhs)
    xl1_d = nc.dram_tensor("xl1", [P, 17], bf16, kind="ExternalInput")
    # w1q: per expert [128p, dc*256+j] = W1[e, c*256+j, dc*128+p]*S1 (e3m4),
    #      cols 4096..4351 = b1 chunk on partition 0 only
    w1q_d = nc.dram_tensor("w1q", [E, P, F1], f8, kind="ExternalInput")
    # w2q: per expert [128p, ic*2048+o] = W2[e, o, c*256+ic*128+p]*S2
    w2q_d = nc.dram_tensor("w2q", [E, P, F2], w2dt, kind="ExternalInput")
    # b2pk: [e, o] = b2[e, o] * S2 / NCORES (expert dim on partitions)
    b2pk_d = nc.dram_tensor("b2pk", [E, H], f32, kind="ExternalInput")
    out_d = nc.dram_tensor("out", [P, OC], f32, kind="ExternalOutput")

    in_names = ["gpbf", "gpf", "xl1", "w1q", "w2q", "b2pk"]

    with tile.TileContext(nc) as tc:
        with (
            tc.tile_pool(name="sb", bufs=1) as sb,
            tc.tile_pool(name="ps", bufs=1, space="PSUM") as ps,
        ):
            # ----- static loads --------------------------------------------
            gp_bf = sb.tile([P, 288], bf16, tag="gp_bf")
            nc.sync.dma_start(gp_bf[:], gpbf_d.ap())  # SP: critical path
            gp_f = sb.tile([P, 289], f32, tag="gp_f")
            nc.scalar.dma_start(gp_f[:], gpf_d.ap())
            xl1 = sb.tile([P, 17], bf16, tag="xl1")
            nc.scalar.dma_start(xl1[:], xl1_d.ap())
            # all experts' scaled b2, expert dim on partitions (static)
            b2pk = sb.tile([E, H], f32, tag="b2pk")
            nc.scalar.dma_start(b2pk[:], b2pk_d.ap())

            one_bf = sb.tile([1, 1], bf16, tag="one_bf")
            nc.gpsimd.memset(one_bf[:], 1.0)
            one_f = sb.tile([1, 1], f32, tag="one_f")
            nc.gpsimd.memset(one_f[:], 1.0)
            ones_col = sb.tile([1, P], f32, tag="ones_col")
            nc.gpsimd.memset(ones_col[:], 1.0)

            # ----- gating fast path (bf16) on PE ---------------------------
            lg_bf = ps.tile([1, E], f32, tag="lg_bf")
            for dc in range(DC):
                nc.tensor.matmul(
                    out=lg_bf[:],
                    lhsT=gp_bf[:, dc : dc + 1],
                    rhs=gp_bf[:, 16 + dc * E : 16 + (dc + 1) * E],
                    start=(dc == 0),
                    stop=False,
                )
            nc.tensor.matmul(
                out=lg_bf[:], lhsT=one_bf[:], rhs=gp_bf[0:1, 272:288],
                start=False, stop=True,
            )

            # top-2 indices in two DVE ops
            vmax_bf = sb.tile([1, 8], f32, tag="vmax_bf")
            nc.vector.max(vmax_bf[:], lg_bf[:])
            vidx = sb.tile([1, 8], u32, tag="vidx")
            nc.vector.max_index(vidx[:], vmax_bf[:], lg_bf[:])

            # ----- idx -> SP registers -> dynamic weight DMAs ---------------
            sv = []
            for k in range(2):
                reg = nc.sync.alloc_register(f"idx_sp{k}")
                nc.sync.reg_load(reg, vidx[0:1, k : k + 1])
                sv.append(nc.snap(reg, donate=True, min_val=0, max_val=E - 1))

            w1t = [
                sb.tile([P, F1], f8, tag=f"w1t{k}", name=f"w1t{k}")
                for k in range(2)
            ]
            w2t = [
                sb.tile([P, F2], w2dt, tag=f"w2t{k}", name=f"w2t{k}")
                for k in range(2)
            ]
            for k in range(2):
                nc.sync.dma_start(
                    w1t[k][:],
                    w1q_d.ap()[bass.ds(sv[k], 1), :, :].rearrange(
                        "o p f -> p (o f)"
                    ),
                )
            nc.sync.dma_start(
                w2t[0][:],
                w2q_d.ap()[bass.ds(sv[0], 1), :, :].rearrange(
                    "o p f -> p (o f)"
                ),
            )
            # e1's W2 split so the last DMA chunk (oc 14..15 of ic 1) is small
            nc.sync.dma_start(
                w2t[1][:, 0:F2_SPLIT],
                w2q_d.ap()[bass.ds(sv[1], 1), :, 0:F2_SPLIT].rearrange(
                    "o p f -> p (o f)"
                ),
            )
            nc.sync.dma_start(
                w2t[1][:, F2_SPLIT:F2],
                w2q_d.ap()[bass.ds(sv[1], 1), :, F2_SPLIT:F2].rearrange(
                    "o p f -> p (o f)"
                ),
            )

            # ----- gating slow path (fp32) for exact gate values ------------
            lg_f = ps.tile([1, E], f32, tag="lg_f")
            for dc in range(DC):
                nc.tensor.matmul(
                    out=lg_f[:],
                    lhsT=gp_f[:, dc : dc + 1],
                    rhs=gp_f[:, 16 + dc * E : 16 + (dc + 1) * E],
                    start=(dc == 0),
                    stop=False,
                )
            nc.tensor.matmul(
                out=lg_f[:], lhsT=one_f[:], rhs=gp_f[0:1, 272:288],
                start=False, stop=True,
            )

            # tkg'_k = tkg_k / S2, with tkg = top2(softmax)/(sum+1e-6)
            vmax_f = sb.tile([1, 8], f32, tag="vmax_f")
            nc.vector.max(vmax_f[:], lg_f[:])
            negm1 = sb.tile([1, 1], f32, tag="negm1")
            nc.vector.tensor_scalar_mul(negm1[:], vmax_f[0:1, 0:1], -1.0)
            esb = sb.tile([1, E], f32, tag="esb")
            nc.scalar.activation(esb[:], lg_f[:], ACTF.Exp, bias=negm1[:])
            e2x = sb.tile([1, 1], f32, tag="e2x")
            nc.scalar.activation(
                e2x[:], vmax_f[0:1, 1:2], ACTF.Exp, bias=negm1[:]
            )
            ssum = sb.tile([1, 1], f32, tag="ssum")
            nc.vector.tensor_reduce(ssum[:], esb[:], axis=AX, op=OP.add)
            # den2 = S2 * (1 + e2x + 1e-6*ssum)
            den2 = sb.tile([1, 1], f32, tag="den2")
            nc.vector.tensor_scalar(
                den2[:], ssum[:], 1e-6 * S2, S2, OP.mult, OP.add
            )
            e2xs = sb.tile([1, 1], f32, tag="e2xs")
            nc.vector.tensor_scalar_mul(e2xs[:], e2x[:], S2)
            nc.vector.tensor_add(den2[:], den2[:], e2xs[:])
            tkgp = [
                sb.tile([1, 1], f32, tag=f"tkgp{k}", name=f"tkgp{k}")
                for k in range(2)
            ]
            nc.vector.reciprocal(tkgp[0][:], den2[:])
            nc.vector.tensor_mul(tkgp[1][:], e2x[:], tkgp[0][:])

            # broadcast tkg' to all partitions via K=1 matmuls
            tkgrep = ps.tile([P, 2], f32, tag="tkgrep")
            for k in range(2):
                nc.tensor.matmul(
                    out=tkgrep[:, k : k + 1],
                    lhsT=ones_col[:],
                    rhs=tkgp[k][:],
                    start=True,
                    stop=True,
                )

            # gsel[e] = sum_k tkg'_k * [e == idx_k]  (16-partition one-hot mix
            # vector; folds the b2 bias into the layer-2 PSUM accumulation)
            idxf = sb.tile([1, 2], f32, tag="idxf")
            nc.vector.tensor_copy(idxf[:], vidx[0:1, 0:2])
            idx_ps = ps.tile([E, 2], f32, tag="idx_ps")
            for k in range(2):
                nc.tensor.matmul(
                    out=idx_ps[:, k : k + 1],
                    lhsT=ones_col[0:1, 0:E],
                    rhs=idxf[0:1, k : k + 1],
                    start=True,
                    stop=True,
                )
            mk = [
                sb.tile([E, 1], f32, tag=f"mk{k}", name=f"mk{k}")
                for k in range(2)
            ]
            for k in range(2):
                nc.vector.tensor_tensor(
                    out=mk[k][:],
                    in0=gp_f[0:E, 288:289],
                    in1=idx_ps[:, k : k + 1],
                    op=OP.is_equal,
                )
                nc.vector.tensor_scalar(
                    mk[k][:], mk[k][:], tkgrep[0:E, k : k + 1], None, OP.mult
                )
            gsel = sb.tile([E, 1], f32, tag="gsel")
            nc.vector.tensor_add(gsel[:], mk[0][:], mk[1][:])

            # ----- layer 1: h = tanh(W1[e] @ x + b1[e]), scaled by tkg' -----
            h_ps = [
                ps.tile([P, NCH], f32, tag=f"hps{k}", name=f"hps{k}")
                for k in range(2)
            ]
            hs = [
                sb.tile([P, NCH], bf16, tag=f"hs{k}", name=f"hs{k}")
                for k in range(2)
            ]
            for k in range(2):
                for rc in range(NCH):
                    for dc in range(DC):
                        nc.tensor.matmul(
                            out=h_ps[k][:, rc : rc + 1],
                            lhsT=w1t[k][
                                :, dc * RS + rc * P : dc * RS + (rc + 1) * P
                            ],
                            rhs=xl1[:, dc : dc + 1],
                            start=(dc == 0),
                            stop=False,
                        )
                    # b1 chunk (partition 0 only) x 1.0
                    nc.tensor.matmul(
                        out=h_ps[k][:, rc : rc + 1],
                        lhsT=w1t[k][
                            :, DC * RS + rc * P : DC * RS + (rc + 1) * P
                        ],
                        rhs=xl1[:, 16:17],
                        start=False,
                        stop=True,
                    )
                for rc in range(NCH):
                    nc.scalar.activation(
                        hs[k][:, rc : rc + 1],
                        h_ps[k][:, rc : rc + 1],
                        ACTF.Tanh,
                    )
                nc.vector.tensor_scalar(
                    hs[k][:], hs[k][:], tkgrep[:, k : k + 1], None, OP.mult
                )

            # ----- layer 2: eo_k = W2[e_k] @ hs_k (gate-weighted); the b2
            # bias term (gsel-weighted mix over all experts) accumulates
            # into expert 0's PSUM between its ic chunks -------------------
            eo_ps = [
                ps.tile([P, OC], f32, tag=f"eops{k}", name=f"eops{k}")
                for k in range(2)
            ]
            for k in range(2):
                for oc in range(OC):
                    nc.tensor.matmul(
                        out=eo_ps[k][:, oc : oc + 1],
                        lhsT=w2t[k][:, oc * P : (oc + 1) * P],
                        rhs=hs[k][:, 0:1],
                        start=True,
                        stop=False,
                    )
                    if k == 0:
                        nc.tensor.matmul(
                            out=eo_ps[k][:, oc : oc + 1],
                            lhsT=b2pk[:, oc * P : (oc + 1) * P],
                            rhs=gsel[:],
                            start=False,
                            stop=False,
                        )
                    nc.tensor.matmul(
                        out=eo_ps[k][:, oc : oc + 1],
                        lhsT=w2t[k][:, H + oc * P : H + (oc + 1) * P],
                        rhs=hs[k][:, 1:2],
                        start=False,
                        stop=True,
                    )

            # ----- combine: res = (eo0 + b2mix) + eo1 -----------------------
            ea0 = sb.tile([P, OC], f32, tag="ea0")
            nc.vector.tensor_copy(ea0[:], eo_ps[0][:])
            res = sb.tile([P, OC], f32, tag="res")
            nc.vector.tensor_tensor(
                out=res[:], in0=eo_ps[1][:], in1=ea0[:], op=OP.add
            )
            nc.sync.dma_start(out_d.ap(), res[:])

    nc.compile()
    _BUILT = (nc, in_names)
    return _BUILT


def make_in_maps(x, Wg, bg, W1, b1, W2, b2):
    """Host-side packing/quantization: per-core input dicts."""
    bf = ml_dtypes.bfloat16
    f8 = ml_dtypes.float8_e3m4
    x = np.asarray(x, np.float32).reshape(D)
    Wg = np.asarray(Wg, np.float32)
    bg = np.asarray(bg, np.float32).reshape(E)
    W1 = np.asarray(W1, np.float32)
    b1 = np.asarray(b1, np.float32)
    W2 = np.asarray(W2, np.float32)
    b2 = np.asarray(b2, np.float32)

    # gating pack: x_pd | WgT | bg(row 0) | partition-iota (f32 only)
    gp = np.zeros((P, 289), np.float32)
    gp[:, 0:DC] = x.reshape(DC, P).T
    gp[:, 16 : 16 + DC * E] = (
        Wg.T.reshape(DC, P, E).transpose(1, 0, 2).reshape(P, DC * E)
    )
    gp[0, 272 : 272 + E] = bg
    gp[0:E, 288] = np.arange(E, dtype=np.float32)
    gpbf = np.ascontiguousarray(gp[:, 0:288]).astype(bf)

    xl1 = np.zeros((P, 17), np.float32)
    xl1[:, 0:DC] = x.reshape(DC, P).T / S1
    xl1[:, 16] = 1.0
    xl1 = xl1.astype(bf)

    w2np = bf if W2_BF16 else f8

    in_maps = []
    for c in range(NCORES):
        rs = slice(c * RS, (c + 1) * RS)
        # W1 slice + bias chunk
        w1s = (
            (W1[:, rs, :] * S1)
            .reshape(E, RS, DC, P)
            .transpose(0, 3, 2, 1)
            .reshape(E, P, DC * RS)
        )
        w1b = np.zeros((E, P, NCH * P), np.float32)
        w1b[:, 0, :] = b1[:, rs]
        w1q = np.ascontiguousarray(
            np.concatenate([w1s, w1b], axis=2)
        ).astype(f8)
        # W2 slice
        w2q = np.ascontiguousarray(
            (W2[:, :, rs] * S2)
            .reshape(E, H, NCH, P)
            .transpose(0, 3, 2, 1)
            .reshape(E, P, NCH * H)
        ).astype(w2np)
        b2pk = np.ascontiguousarray(b2 * (S2 / NCORES))
        in_maps.append(
            {
                "gpbf": gpbf,
                "gpf": gp,
                "xl1": xl1,
                "w1q": w1q,
                "w2q": w2q,
                "b2pk": b2pk,
            }
        )
    return in_maps


def kernel(x, Wg, bg, W1, b1, W2, b2, train=0, **_unused):
    from concourse import bass_utils

    nc, _ = _build()
    in_maps = make_in_maps(x, Wg, bg, W1, b1, W2, b2)
    res = bass_utils.run_bass_kernel_spmd(
        nc, in_maps, core_ids=list(range(NCORES))
    )
    outs = [
        np.asarray(res.results[c]["out"], np.float32).reshape(P, OC)
        for c in range(NCORES)
    ]
    # each core holds a gate-weighted partial over its contraction shard;
    # out[oc*128+p] = sum_c outs[c][p, oc]
    tot = np.sum(outs, axis=0, dtype=np.float32)
    return tot.T.reshape(H).copy()
